# revision 75
# baseline (speedup 1.0000x reference)
"""Trainium2 Bass kernel for nn_DEAM_79044578116356 (dilated 9-neighbor local
attention block: conv1x1+BN+ReLU -> qkv -> 3x3 dil-2 neighborhood softmax
attention -> residual -> 1x1 fc).

Contract: kernel(**inputs) takes the FULL unsharded inputs (B=8) and returns
the FULL [8, 64, 128, 128] float32 output. Internally shards data-parallel
over batch across the 8 NeuronCores (weights replicated), one image per core.

Device layout (per core): partition = c + 64*(h%2), free = rp*W + w with
rp = h//2. dy shifts in {-2,0,2} preserve row parity, so every dilated
(dy,dx) shift of k/v is a pure free-dim offset into a zero-padded
[66 rp x 132 w] plane.

Structure (all matmuls fp16; elementwise fp16 wherever possible):
 - conv: fp16 matmuls (e_map cast to fp16 on a gpsimd DMA); BN folded into
   the weights host-side, conv bias applied via the relu evacuation's
   per-partition bias on Act; q/k/v psum ping-pongs through 2 banks.
 - scores: q*k_n products grouped 3-per-op on DVE (dx windows via an
   overlapping-stride raw AP, q broadcast via a stride-0 dim), then one
   matmul per n with a parity-block ones matrix PB that reduces over
   channels AND replicates the score to all 128 PSUM partitions.
 - exp: Act reads replicated scores from PSUM in n-pairs, writes e_n to
   fp16 SBUF (scores are small & non-negative; no max-subtraction needed).
 - z = sum_n e_n: pair-partials on Pool (off the critical path), final
   3 adds + reciprocal on DVE.
 - U = sum_n e_n * v_n: products grouped 3-per-op on DVE, accumulated in
   PSUM via identity matmuls on PE.
 - out = fc(U*zr + f): normalize fused with the U evacuation (DVE), the
   residual enters as a second accumulating fc matmul on PE, fc bias is
   folded into f_map host-side (f + solve(fc_w, fc_b)); fc output
   evacuated once per chunk (Act) and DMA'd to DRAM.
 - 3-stage software pipeline: scores(ch) || AV+z(ch-1) || tail(ch-2),
   conv chunks interleaved one ahead; PSUM = conv 2 + score-pairs 4 +
   U 2 banks.
"""
import os

import numpy as np

os.environ.setdefault("JAX_COMPILATION_CACHE_DIR", "/tmp/jax_neff_cache")

import bass_rust
import concourse.bass as bass
import concourse.mybir as mybir
from concourse.bass_utils import run_bass_kernel_spmd
from concourse.tile import TileContext

# ---------------------------------------------------------------------------
# Workaround for this walrus build's 1-sync-wait-per-instruction limit
# ("Too many sync wait commands" from setupSyncWait for CTRL/S3_LW/...).
# Extra sem waits are hoisted onto same-engine InstNoOp instructions placed
# immediately before the owner (engines run in program order, so an earlier
# same-engine wait is equivalent).
# ---------------------------------------------------------------------------
import concourse.tile as _tile_mod
from concourse.vector_clock import ScopedClock as _ScopedClock

_MAX_WAITS = 1


def _split_inst_waits(nc, inst, out_list):
    si = inst.sync_info
    if si is None or not si.on_wait or len(si.on_wait) <= _MAX_WAITS:
        out_list.append(inst)
        return
    waits = list(si.on_wait)
    keep, extra = waits[:_MAX_WAITS], waits[_MAX_WAITS:]
    si.on_wait.clear()
    si.on_wait.extend(keep)
    for i in range(0, len(extra), _MAX_WAITS):
        chunk = extra[i:i + _MAX_WAITS]
        nop = mybir.InstNoOp(
            name=nc.get_next_instruction_name(),
            engine=inst.engine,
            ins=[],
            outs=[],
            sync_info=mybir.SyncInfo(on_wait=list(chunk), on_update=[]),
            bass_nofuse=True,
        )
        nc.register_instruction(nop, overwrite=True)
        out_list.append(nop)
    out_list.append(inst)


if not getattr(_tile_mod.TileContext, "_deam_wait_patch", False):
    _orig_lower = _tile_mod.TileContext._lower_ordered_insts

    def _patched_lower(self, ordered):
        nc = self.nc
        for _bb, insts in ordered.items():
            new_list = []
            for inst in insts:
                _split_inst_waits(nc, inst, new_list)
            insts[:] = new_list
        return _orig_lower(self, ordered)

    def _patched_drain_and_barrier(self, tick_clock, wait_clock):
        nc = self.nc
        drain_inst = nc.sync.drain()
        wait_clock.add_sem_waits(
            drain_inst.ins, _ScopedClock({None: tick_clock.global_clock})
        )
        inst = drain_inst.ins
        si = inst.sync_info
        if si is not None and si.on_wait and len(si.on_wait) > _MAX_WAITS:
            waits = list(si.on_wait)
            si.on_wait.clear()
            si.on_wait.extend(waits[:_MAX_WAITS])
            rest = waits[_MAX_WAITS:]
            while rest:
                chunk, rest = rest[:_MAX_WAITS], rest[_MAX_WAITS:]
                nop = nc.sync.nop(nofuse=True, hint="drain_wait_split")
                nsi = nop.ins.sync_info
                if nsi is None:
                    nop.ins.sync_info = mybir.SyncInfo(on_wait=list(chunk),
                                                       on_update=[])
                else:
                    nsi.on_wait.extend(chunk)
        nc.all_engine_barrier()
        assert self.sems is not None
        popped = nc._tile_sem_poison_stack.pop()
        assert popped is self._sem_poison
        nc.clear_and_free_semaphores(list(self.sems.allocated().values()))
        nc.all_engine_barrier()

    _tile_mod.TileContext._lower_ordered_insts = _patched_lower
    _tile_mod.TileContext._drain_and_barrier = _patched_drain_and_barrier
    _tile_mod.TileContext._deam_wait_patch = True

# ---------------------------------------------------------------------------
# Problem constants (hardcoded per the harness contract)
# ---------------------------------------------------------------------------
F32 = mybir.dt.float32
F16 = mybir.dt.float16
B = 8
C, H, W = 64, 128, 128
HW = H * W
RP = H // 2            # 64 row-pairs
KW = W + 4             # 132 (w padded by 2 each side)
KR = RP + 2            # 66  (rp padded by 1 each side)
KF = KR * KW
QF = RP * W            # 8192 packed columns per parity-pair layout
BN_EPS = 1e-5
OFFS = [(dy, dx) for dy in (-2, 0, 2) for dx in (-2, 0, 2)]
ACH = 16               # attention chunks (4 rp each, fa=512)
CCH = 16               # conv chunks (8 image rows each)
MULT = mybir.AluOpType.mult
ADD = mybir.AluOpType.add
RELU = mybir.ActivationFunctionType.Relu
EXP = mybir.ActivationFunctionType.Exp


def _host_consts(conv1_w, conv1_b, bn_gamma, bn_beta, bn_mean, bn_var,
                 fc_w, fc_b):
    """Fold BN into the conv and 1/sqrt(C) into the q weights; build the
    constant matrices for the on-device reduce/replicate/accumulate matmuls.
    All matmul operands are fp16."""
    inv = (bn_gamma / np.sqrt(bn_var + BN_EPS)).astype(np.float32)
    Wf = (conv1_w * inv[:, None]).astype(np.float32)          # [192, 64]
    bf = (conv1_b * inv + (bn_beta - bn_mean * inv)).astype(np.float32)
    scale = np.float32(1.0 / np.sqrt(np.float32(C)))
    WQ = Wf[0:64].T * scale
    WK = Wf[64:128].T
    WV = Wf[128:192].T
    # conv biases applied at the relu evacuation (per-partition, parity dup)
    BQ = np.tile(bf[0:64] * scale, 2).reshape(128, 1).astype(np.float32)
    BK = np.tile(bf[64:128], 2).reshape(128, 1).astype(np.float32)
    BV = np.tile(bf[128:192], 2).reshape(128, 1).astype(np.float32)
    # parity-block ones: reduce over the 64 channels of each parity and
    # replicate the sum to every partition of the same parity
    PB = np.zeros((128, 128), np.float32)
    PB[0:64, 0:64] = 1.0
    PB[64:128, 64:128] = 1.0
    I128 = np.eye(128, dtype=np.float32)
    FCE = np.zeros((128, 64), np.float32)
    FCO = np.zeros((128, 64), np.float32)
    FCE[0:64] = fc_w.T.astype(np.float32)
    FCO[64:128] = fc_w.T.astype(np.float32)
    # pack all fp16 constants into one [128, 576] tensor (single DMA):
    # cols 0:64 wq | 64:128 wk | 128:192 wv (rows 0:64 used)
    # 192:320 PB | 320:448 I128 | 448:512 FCE | 512:576 FCO
    CST = np.zeros((128, 576), np.float32)
    CST[0:64, 0:64] = WQ
    CST[0:64, 64:128] = WK
    CST[0:64, 128:192] = WV
    CST[:, 192:320] = PB
    CST[:, 320:448] = I128
    CST[:, 448:512] = FCE
    CST[:, 512:576] = FCO
    BIAS = np.concatenate([BQ, BK, BV], axis=1)   # [128, 3] f32
    return dict(CST=CST.astype(np.float16), BIAS=BIAS.astype(np.float32))


def build(nc: bass.Bass):
    e_map = nc.dram_tensor("e_map", [C, H, W], F32, kind="ExternalInput")
    f_aug = nc.dram_tensor("f_aug", [C, H, W], F32, kind="ExternalInput")
    CST = nc.dram_tensor("CST", [128, 576], F16, kind="ExternalInput")
    BIAS = nc.dram_tensor("BIAS", [128, 3], F32, kind="ExternalInput")
    y = nc.dram_tensor("y", [C, H, W], F32, kind="ExternalOutput")

    with TileContext(nc) as tc:
        with tc.tile_pool(name="persist", bufs=1) as P:
            q2 = P.tile([128, QF], F16, tag="q2")
            k2 = P.tile([128, KF], F16, tag="k2")
            v2 = P.tile([128, KF], F16, tag="v2")
            xf = P.tile([128, QF], F16, tag="xf")       # f_aug, parity packed
            cst = P.tile([128, 576], F16, tag="cst")
            bias = P.tile([128, 3], F32, tag="bias")
            nc.sync.dma_start(bias[:, :], BIAS[:, :])
            nc.sync.dma_start(cst[:, :], CST[:, :])
            wq, wk, wv = cst[0:64, 0:64], cst[0:64, 64:128], cst[0:64, 128:192]
            pb, i128 = cst[:, 192:320], cst[:, 320:448]
            fce, fco = cst[:, 448:512], cst[:, 512:576]
            bq, bk, bv = bias[:, 0:1], bias[:, 1:2], bias[:, 2:3]

            q2r = q2[:, :].rearrange("p (r w) -> p r w", w=W)
            k2r = k2[:, :].rearrange("p (r w) -> p r w", w=KW)
            v2r = v2[:, :].rearrange("p (r w) -> p r w", w=KW)
            xfr = xf[:, :].rearrange("p (r w) -> p r w", w=W)

            # ------------- interleaved conv + attention + fc ---------------
            est = P.tile([64, HW], F16, tag="est")
            estr = est[:, :].rearrange("p (h w) -> p h w", w=W)
            # split the e_map cast-load so conv chunk 0 starts early
            for piece in range(4):
                nc.gpsimd.dma_start(
                    est[:, piece * (HW // 4):(piece + 1) * (HW // 4)],
                    e_map[:, piece * (H // 4):(piece + 1) * (H // 4), :])
            # halo borders of k2/v2 (rp rows 0,65; w cols 0:2,130:132)
            for t in (k2, v2):
                tr = t[:, :].rearrange("p (r w) -> p r w", w=KW)
                nc.gpsimd.memset(tr[:, 0:1, :], 0.0)
                nc.gpsimd.memset(tr[:, KR - 1:KR, :], 0.0)
                nc.gpsimd.memset(tr[:, :, 0:2], 0.0)
                nc.gpsimd.memset(tr[:, :, KW - 2:KW], 0.0)
            # f_aug cast-load (f32 DRAM -> fp16 SBUF), parity packed; only
            # needed by the fc stage so it loads after est
            for par in (0, 1):
                nc.gpsimd.dma_start(
                    xf[64 * par:64 * par + 64, :].rearrange(
                        "p (r w) -> p r w", w=W),
                    f_aug[:, par:H:2, :])

            crp = RP // ACH            # 4 row-pairs per attn chunk
            fa = crp * W               # 512 packed cols
            nrow = H // CCH            # 8 rows per conv chunk
            nr2 = nrow // 2            # 4 row-pairs
            cfa = nr2 * W              # 512 packed cols

            def conv_chunk(CP, ch, v_on_dve=False):
                # two psum banks ping-pong: q->A, k->B, v->A' (after q evac)
                h0 = ch * nrow
                rp0 = h0 // 2
                pq = CP.tile([128, cfa], F32, tag="ca")
                pk = CP.tile([128, cfa], F32, tag="cb")
                for par in (0, 1):
                    rhs = estr[:, h0 + par:h0 + nrow:2, :]
                    ps = slice(64 * par, 64 * par + 64)
                    nc.tensor.matmul(pq[ps, :], wq, rhs,
                                     start=True, stop=True)
                    nc.tensor.matmul(pk[ps, :], wk, rhs,
                                     start=True, stop=True)
                nc.scalar.activation(
                    q2r[:, rp0:rp0 + nr2, :],
                    pq[:, :].rearrange("p (r w) -> p r w", w=W), RELU,
                    bias=bq)
                nc.scalar.activation(
                    k2r[:, rp0 + 1:rp0 + 1 + nr2, 2:2 + W],
                    pk[:, :].rearrange("p (r w) -> p r w", w=W), RELU,
                    bias=bk)
                pv = CP.tile([128, cfa], F32, tag="ca")
                for par in (0, 1):
                    rhs = estr[:, h0 + par:h0 + nrow:2, :]
                    ps = slice(64 * par, 64 * par + 64)
                    nc.tensor.matmul(pv[ps, :], wv, rhs,
                                     start=True, stop=True)
                if v_on_dve:
                    # (psum + bias) then relu on DVE to unload Act early on
                    nc.vector.tensor_scalar(
                        v2r[:, rp0 + 1:rp0 + 1 + nr2, 2:2 + W],
                        pv[:, :].rearrange("p (r w) -> p r w", w=W),
                        bv, 0.0, ADD, mybir.AluOpType.max)
                else:
                    nc.scalar.activation(
                        v2r[:, rp0 + 1:rp0 + 1 + nr2, 2:2 + W],
                        pv[:, :].rearrange("p (r w) -> p r w", w=W), RELU,
                        bias=bv)

            def shift9(t2, rp_base, s):
                """[128, 3(dx), crp, W] overlapping view of a padded plane:
                dx-window stride 2, row stride KW, starting at row rp_base+s."""
                base = t2[:, :]
                return bass_rust.AP(
                    tensor=base.tensor, offset=(rp_base + s) * KW,
                    ap=[[KF, 128], [2, 3], [KW, crp], [1, W]])

            def attn_scores(SPS, E9P, PRD, ch):
                # grouped qk products (3 dx per op), reduce-replicate score
                # matmuls, paired exps
                rp0 = ch * crp
                e9 = E9P.tile([128, 9 * fa], F16, tag="e9")
                qb = q2r[:, rp0:rp0 + crp, :].unsqueeze(1).broadcast_to(
                    [128, 3, crp, W])
                prods = []
                for s in range(3):
                    prod3 = PRD.tile([128, 3 * fa], F16, tag="prod")
                    nc.vector.tensor_tensor(
                        prod3[:, :].rearrange("p (x r w) -> p x r w",
                                              x=3, w=W),
                        qb, shift9(k2, rp0, s), MULT)
                    prods.append(prod3)
                s_ps = None
                for n in range(9):
                    half = n % 2
                    if half == 0:
                        s_ps = SPS.tile([128, 2 * fa], F32, tag="s")
                    nc.tensor.matmul(
                        s_ps[:, half * fa:(half + 1) * fa], pb,
                        prods[n // 3][:, (n % 3) * fa:(n % 3 + 1) * fa],
                        start=True, stop=True)
                    if half == 1:
                        nc.scalar.activation(e9[:, (n - 1) * fa:(n + 1) * fa],
                                             s_ps[:, :], EXP)
                    elif n == 8:
                        nc.scalar.activation(e9[:, 8 * fa:9 * fa],
                                             s_ps[:, 0:fa], EXP)
                return e9

            def attn_av(SPS, UPS, WNP, ZZP, ch, e9):
                # grouped e*v products, U psum accumulation, z partials
                rp0 = ch * crp
                u_ps = UPS.tile([128, fa], F32, tag="u")
                for s in range(3):
                    wn3 = WNP.tile([128, 3 * fa], F16, tag="wn")
                    nc.vector.tensor_tensor(
                        wn3[:, :].rearrange("p (x r w) -> p x r w", x=3, w=W),
                        e9[:, 3 * s * fa:(3 * s + 3) * fa].rearrange(
                            "p (x r w) -> p x r w", x=3, w=W),
                        shift9(v2, rp0, s), MULT)
                    for j in range(3):
                        n = 3 * s + j
                        nc.tensor.matmul(u_ps[:, :], i128,
                                         wn3[:, j * fa:(j + 1) * fa],
                                         start=(n == 0), stop=(n == 8))
                # z partials: (e0+e1, e2+e3) and (e4+e5, e6+e7) on Pool,
                # then pair-collapses split Pool/DVE
                a1 = ZZP.tile([128, 2 * fa], F16, tag="a1")
                a2 = ZZP.tile([128, 2 * fa], F16, tag="a2")
                b1 = ZZP.tile([128, fa], F16, tag="b1")
                b2 = ZZP.tile([128, fa], F16, tag="b2")
                e4v = e9[:, 0:8 * fa].rearrange("p (x f) -> p x f", f=2 * fa)
                nc.gpsimd.tensor_tensor(
                    a1[:, :].rearrange("p (x f) -> p x f", f=fa),
                    e4v[:, 0:2, 0:fa], e4v[:, 0:2, fa:2 * fa], ADD)
                nc.gpsimd.tensor_tensor(
                    a2[:, :].rearrange("p (x f) -> p x f", f=fa),
                    e4v[:, 2:4, 0:fa], e4v[:, 2:4, fa:2 * fa], ADD)
                nc.gpsimd.tensor_tensor(b1[:, :], a1[:, 0:fa],
                                        a1[:, fa:2 * fa], ADD)
                nc.vector.tensor_tensor(b2[:, :], a2[:, 0:fa],
                                        a2[:, fa:2 * fa], ADD)
                nc.vector.tensor_tensor(b1[:, :], b1[:, :], b2[:, :], ADD)
                return u_ps, b1

            def attn_tail(SPS, XQP, ZZP, ch, e9, u_ps, zsum):
                # z final, 1/z, normalize, fc, output
                rp0 = ch * crp
                z1 = ZZP.tile([128, fa], F16, tag="z1")
                zr = ZZP.tile([128, fa], F16, tag="zr")
                nc.vector.tensor_tensor(z1[:, :], zsum[:, :],
                                        e9[:, 8 * fa:9 * fa], ADD)
                with nc.allow_low_precision(reason="z in [9, ~3e4]; fp16 "
                                            "recip rel err ~5e-4"):
                    nc.vector.reciprocal(zr[:, :], z1[:, :])
                # xq = U * zr  (residual comes in via the fc accumulation)
                xq = XQP.tile([128, fa], F16, tag="xq")
                nc.vector.tensor_tensor(xq[:, :], u_ps[:, :], zr[:, :],
                                        MULT)
                # fc over (xq + f): accumulate fc(f) then fc(xq) in PSUM,
                # both parities stacked in one tile of the scores ring
                fc_pair = SPS.tile([128, 2 * fa], F32, tag="s")
                fc_ps = fc_pair[:, 0:fa]
                xfc = xf[:, rp0 * W:(rp0 + crp) * W]
                for ps, fcw in ((slice(0, 64), fce), (slice(64, 128), fco)):
                    nc.tensor.matmul(fc_ps[ps, :], fcw, xfc,
                                     start=True, stop=False)
                    nc.tensor.matmul(fc_ps[ps, :], fcw, xq[:, :],
                                     start=False, stop=True)
                ob = XQP.tile([128, fa], F32, tag="ob")
                nc.scalar.activation(ob[:, :], fc_ps[:, :],
                                     mybir.ActivationFunctionType.Copy)
                for par in (0, 1):
                    nc.sync.dma_start(
                        y[:, 2 * rp0 + par:2 * (rp0 + crp):2, :],
                        ob[64 * par:64 * par + 64, :].rearrange(
                            "p (r w) -> p r w", w=W))

            # PSUM budget (8 banks): conv A+B 2, s pairs 2x2, u 2x1
            with tc.tile_pool(name="cps", bufs=1, space="PSUM") as CP, \
                 tc.tile_pool(name="sps", bufs=2, space="PSUM") as SPS, \
                 tc.tile_pool(name="ups", bufs=2, space="PSUM") as UPS, \
                 tc.tile_pool(name="e9p", bufs=4) as E9P, \
                 tc.tile_pool(name="prd", bufs=6) as PRD, \
                 tc.tile_pool(name="wnp", bufs=4) as WNP, \
                 tc.tile_pool(name="zzp", bufs=3) as ZZP, \
                 tc.tile_pool(name="xqp", bufs=3) as XQP:
                # software pipeline: scores(ch) || AV+z(ch-1) || tail(ch-2)
                conv_chunk(CP, 0, v_on_dve=True)
                conv_chunk(CP, 1, v_on_dve=True)
                state = {}  # ch -> (e9, u_ps, zsum)
                e9_prev = attn_scores(SPS, E9P, PRD, 0)
                for ch in range(1, ACH):
                    if ch + 1 < CCH:
                        conv_chunk(CP, ch + 1)
                    e9 = attn_scores(SPS, E9P, PRD, ch)
                    u_ps, zs = attn_av(SPS, UPS, WNP, ZZP, ch - 1, e9_prev)
                    state[ch - 1] = (e9_prev, u_ps, zs)
                    e9_prev = e9
                    if ch - 2 in state:
                        pe9, pu, pz = state.pop(ch - 2)
                        attn_tail(SPS, XQP, ZZP, ch - 2, pe9, pu, pz)
                u_ps, zs = attn_av(SPS, UPS, WNP, ZZP, ACH - 1, e9_prev)
                state[ACH - 1] = (e9_prev, u_ps, zs)
                for ch in (ACH - 2, ACH - 1):
                    pe9, pu, pz = state.pop(ch)
                    attn_tail(SPS, XQP, ZZP, ch, pe9, pu, pz)
    return nc


_build_cache = {}


def _get_nc():
    if "nc" not in _build_cache:
        nc = bass.Bass()
        build(nc)
        _build_cache["nc"] = nc
    return _build_cache["nc"]


def run_spmd(in_maps, **kw):
    """Run the prebuilt program on cores 0..len(in_maps)-1."""
    nc = _get_nc()
    return run_bass_kernel_spmd(nc, in_maps, core_ids=list(range(len(in_maps))),
                                **kw)


def make_in_maps(f_map, e_map, conv1_w, conv1_b, bn_gamma, bn_beta, bn_mean,
                 bn_var, fc_w, fc_b):
    consts = _host_consts(np.asarray(conv1_w), np.asarray(conv1_b),
                          np.asarray(bn_gamma), np.asarray(bn_beta),
                          np.asarray(bn_mean), np.asarray(bn_var),
                          np.asarray(fc_w), np.asarray(fc_b))
    f_map = np.ascontiguousarray(np.asarray(f_map, dtype=np.float32))
    e_map = np.ascontiguousarray(np.asarray(e_map, dtype=np.float32))
    fc_w = np.asarray(fc_w, dtype=np.float32)
    fc_b = np.asarray(fc_b, dtype=np.float32)
    # fold the fc bias into the residual input: fc(x + c) = fc(x) + fc_b
    # with c = solve(fc_w, fc_b) (einsum 'oc' convention: fc_w @ c = fc_b)
    if np.any(fc_b):
        c = np.linalg.solve(fc_w, fc_b).astype(np.float32)
        f_aug = f_map + c[None, :, None, None]
    else:
        f_aug = f_map
    return [dict(e_map=e_map[b], f_aug=f_aug[b], **consts) for b in range(B)]


def kernel(f_map, e_map, conv1_w, conv1_b, bn_gamma, bn_beta, bn_mean, bn_var,
           fc_w, fc_b):
    in_maps = make_in_maps(f_map, e_map, conv1_w, conv1_b, bn_gamma, bn_beta,
                           bn_mean, bn_var, fc_w, fc_b)
    res = run_spmd(in_maps)
    out = np.stack([res.results[b]["y"] for b in range(B)]).astype(np.float32)
    return out
